# revision 11
# baseline (speedup 1.0000x reference)
"""Causal self-attention (B=4, T=2048, C=1024, H=16) on 8 trn2 NeuronCores.

Sharding: tensor-parallel over heads (2 heads/core) for QKV projection +
attention; output rows are resharded core-cyclically by 128-row tiles
(dest core d owns row-tiles t with t % 8 == d) so each batch gets its own
quarter-size AllToAll that serves all 8 cores symmetrically — the
collective for batch b fires as soon as batch b's attention is done and
overlaps batch b+1's compute.  Host gather interleaves the tiles back.

Key PE optimizations over the plain per-head schedule:
  * QK^T is ROW-TILED: head 0 lives on partitions 0-63 of the qT/kT
    slabs, head 1 on 64-127.  Each k-tile issues two 64-contraction
    matmuls at tile_position (0,0) and (64,0) which execute
    CONCURRENTLY on the two halves of the PE array (~2x QK throughput).
    Scores share one [128,1024] PSUM group (h0 cols 0:512, h1 512:1024)
    so a single ACT exp instruction covers both heads.
  * V is computed directly in [row, oc] layout (x-stationary matmuls)
    — no PE transposes, no vst staging copy.
  * V slots are packed [v_h0(64) | ones | v_h1(64) | ones] so the PV
    matmul of either head is a 65-wide lhsT whose output carries y in
    partitions 0-63 and the softmax denominator in partition 64 (no
    max-subtraction: logits are small).  No zero padding needed.
  * c_proj bias is applied by the DVE during the PSUM->SBUF copy
    (tensor_tensor add against a gpsimd-broadcast bias tile) instead of
    a PE matmul.
The causal mask stays a -30 additive bias via PE matmul accumulation
(identity x mask-bias) on the two diagonal-tile column blocks.
The normalizer 1/den is partition-broadcast on gpsimd; y/den are copied
out of PSUM in one [65,512] DVE op so the bank frees early.

PSUM budget (8 banks): score groups 2x2 + y accum 2x1 + shared
qkv/proj accumulator 2x1.
"""

import sys

for _p in ("/opt/trn_rl_repo",):
    if _p not in sys.path:
        sys.path.insert(0, _p)

import numpy as np
import ml_dtypes

B, T, C, H, HS = 4, 2048, 1024, 16, 64
NCORES = 8
HPC = H // NCORES            # heads per core = 2
CPC = HPC * HS               # channels per core = 128
ROWS = B * T                 # 8192
RPC = ROWS // NCORES         # rows per core = 1024
NKT = T // 128               # k-tiles per batch = 16

BF16 = ml_dtypes.bfloat16

_CACHE: dict = {}


def _apply_tile_tail_patch(tile_mod):
    """This container's walrus rejects CTRL-class instructions (Drain/NoOp)
    carrying semaphore waits. Re-emit TileContext's tail waits as individual
    EventSemaphore waits and use the sem-only barrier variant."""
    import bass_rust
    from concourse.vector_clock import ScopedClock

    if getattr(tile_mod.TileContext, "_tail_patch_applied", False):
        return

    def _drain_and_barrier(self, tick_clock, wait_clock):
        collector = self.nc.sync.nop(nofuse=True, hint="tile_tail_wait")
        wait_clock.add_sem_waits(
            collector.ins, ScopedClock({None: tick_clock.global_clock})
        )
        si = collector.ins.sync_info
        waits = list(si.on_wait) if si is not None else []
        collector.ins.sync_info = None
        for w in waits:
            assert w.wait_mode == "sem-ge-imm", w
            self.nc.sync.wait_ge(
                bass_rust.SemaphoreHandle(w.ant_name, w.id), w.wait_value
            )

        self.nc.all_engine_barrier(sem_only=True)
        assert self.sems is not None
        popped = self.nc._tile_sem_poison_stack.pop()
        assert popped is self._sem_poison
        self.nc.clear_and_free_semaphores(list(self.sems.allocated().values()))
        self.nc.all_engine_barrier(sem_only=True)

    tile_mod.TileContext._drain_and_barrier = _drain_and_barrier
    tile_mod.TileContext._tail_patch_applied = True


def _build(dbg=False):
    import concourse.bass as bass
    import concourse.bacc as bacc
    import concourse.mybir as mybir
    import concourse.tile as tile

    _apply_tile_tail_patch(tile)

    dt = mybir.dt
    F32 = dt.float32
    BF = dt.bfloat16
    Exp = mybir.ActivationFunctionType.Exp

    nc = bacc.Bacc(num_devices=NCORES)

    if dbg:
        dbg_q = nc.dram_tensor("dbg_q", [128, ROWS], BF, kind="ExternalOutput")
        dbg_k = nc.dram_tensor("dbg_k", [128, ROWS], BF, kind="ExternalOutput")
        dbg_vp = nc.dram_tensor("dbg_vp", [128, NKT * B * 256], BF, kind="ExternalOutput")
        dbg_ycp = nc.dram_tensor("dbg_ycp", [65, 32, 512], F32, kind="ExternalOutput")
        dbg_bpc = nc.dram_tensor("dbg_bpc", [128, C], F32, kind="ExternalOutput")
        dbg_ain = nc.dram_tensor("dbg_ain", [NCORES * CPC, 256], BF, kind="ExternalOutput")
        dbg_aout = nc.dram_tensor("dbg_aout", [NCORES * CPC, 256], BF, kind="ExternalOutput")
        dbg_yth = nc.dram_tensor("dbg_yth", [128, 8 * 256], BF, kind="ExternalOutput")

    xT = nc.dram_tensor("xT", [C, ROWS], BF, kind="ExternalInput")
    wqk = nc.dram_tensor("wqk", [C, 2 * CPC], BF, kind="ExternalInput")
    wv = nc.dram_tensor("wv", [C, CPC], BF, kind="ExternalInput")
    bq = nc.dram_tensor("bq", [CPC, 1], F32, kind="ExternalInput")
    bk = nc.dram_tensor("bk", [CPC, 1], F32, kind="ExternalInput")
    wp = nc.dram_tensor("wp", [C, C], BF, kind="ExternalInput")
    bprime = nc.dram_tensor("bprime", [1, C], F32, kind="ExternalInput")
    tril = nc.dram_tensor("tril", [128, 128], BF, kind="ExternalInput")
    out = nc.dram_tensor("out", [RPC, C], F32, kind="ExternalOutput")

    with tile.TileContext(nc) as tc:
        with (
            tc.tile_pool(name="const", bufs=1) as constp,
            tc.tile_pool(name="big", bufs=1) as bigp,
            tc.tile_pool(name="xin", bufs=3) as xinp,
            tc.tile_pool(name="pt", bufs=5) as ptp,
            tc.tile_pool(name="tail", bufs=4) as tailp,
            tc.tile_pool(name="osb", bufs=2) as osbp,
            tc.tile_pool(name="yth", bufs=2) as ythp,
            tc.tile_pool(name="psy", bufs=2, space="PSUM") as psy,
            tc.tile_pool(name="psst", bufs=2, space="PSUM") as psst,
            tc.tile_pool(name="psmm", bufs=2, space="PSUM") as psmm,
            tc.tile_pool(name="dram", bufs=1, space="DRAM") as dramp,
        ):
            # ---- constants (only what the first r-chunk needs up front;
            # the rest is issued after the first x tile DMA so PE starts
            # ~20us earlier) ----
            # split per contraction-tile so the transfers round-robin over
            # DMA queues and the first q matmul starts ~3us in
            wqk_sb = constp.tile([128, 8, 2 * CPC], BF, tag="wqk")
            wqk_r = wqk.rearrange("(ct p) o -> p ct o", p=128)
            for ct in range(8):
                nc.sync.dma_start(wqk_sb[:, ct, :], wqk_r[:, ct, :])
            bq_sb = constp.tile([CPC, 1], F32, tag="bq")
            nc.sync.dma_start(bq_sb[:], bq[:])
            bk_sb = constp.tile([CPC, 1], F32, tag="bk")
            nc.sync.dma_start(bk_sb[:], bk[:])
            wv_sb = constp.tile([128, 8, CPC], BF, tag="wv")
            wv_r = wv.rearrange("(ct p) o -> p ct o", p=128)
            for ct in range(8):
                nc.sync.dma_start(wv_sb[:, ct, :], wv_r[:, ct, :])
            wp_sb = constp.tile([128, 8, C], BF, tag="wp")
            bprime_sb = constp.tile([1, C], F32, tag="bprime")
            bprime_bc = constp.tile([128, C], F32, tag="bprime_bc")
            tril_sb = constp.tile([128, 128], BF, tag="tril")
            nc.sync.dma_start(tril_sb[:], tril[:])

            # dummy broadcast: forces the gpsimd Q7 library resident before
            # the first real per-chunk broadcast (~15us in) — the library
            # load otherwise races it on the first execution in a process
            warm_src = constp.tile([1, 512], F32, tag="wsrc")
            nc.vector.memset(warm_src[:], 1.0)
            warm_dst = constp.tile([64, 512], F32, tag="wdst")
            nc.gpsimd.partition_broadcast(warm_dst[:], warm_src[:])

            def late_consts():
                nc.sync.dma_start(bprime_sb[:], bprime[:])
                nc.gpsimd.partition_broadcast(bprime_bc[:], bprime_sb[:])
                nc.sync.dma_start(
                    wp_sb[:], wp.rearrange("(ct p) o -> p ct o", p=128)
                )

            # ---- persistent intermediates ----
            # qT/kT: [128, ROWS] — head 0 channels on partitions 0-63,
            # head 1 on 64-127 (the row-tiling layout).
            qT_sb = bigp.tile([128, ROWS], BF, tag="qT")
            kT_sb = bigp.tile([128, ROWS], BF, tag="kT")
            # v' per global k-tile: [128 rows, 64 slots, 2*128]; per
            # head half: [v 64 | ones | 63 zeros] (the 65-col lhsT read
            # produced garbage on hw; 128-col weights are the safe path).
            vp_sb = bigp.tile([128, NKT * B, 256], BF, tag="vp")
            vpr = vp_sb[:].rearrange("p s (h c) -> p s h c", c=128)
            nc.vector.memset(vpr[:, :, :, 64:65], 1.0)
            nc.vector.memset(vpr[:, :, :, 65:128], 0.0)

            # per-batch AllToAll buffers: [8 dests x 128ch, 256 q]; dest
            # core d owns the contiguous 256-row block d of each batch, so
            # every tail's scatter lands in 512B-contiguous runs (the
            # 128-row-cyclic layout halved the DMA run length)
            a2a_in = [
                dramp.tile([NCORES * CPC, 256], BF, name=f"a2a_in{b}")
                for b in range(B)
            ]
            a2a_out = [
                dramp.tile([NCORES * CPC, 256], BF, name=f"a2a_out{b}")
                for b in range(B)
            ]

            xT_r = xT.rearrange("(ct p) r -> p ct r", p=128)

            def qkv_r(b, rb):
                with nc.named_scope(f"qkv{b}{rb}"):
                    _qkv_r(b, rb)

            def _qkv_r(b, rb):
                r = b * 4 + rb
                rs = slice(r * 512, (r + 1) * 512)
                xt = xinp.tile([128, 8, 512], BF, tag="xt")
                for cth in range(4):
                    nc.sync.dma_start(
                        xt[:, 2 * cth : 2 * cth + 2, :],
                        xT_r[:, 2 * cth : 2 * cth + 2, rs],
                    )

                q_ps = psmm.tile([128, 512], F32, tag="mm", name=f"qps_{r}")
                for ct in range(8):
                    nc.tensor.matmul(
                        q_ps[:], wqk_sb[:, ct, 0:CPC], xt[:, ct, :],
                        start=(ct == 0), stop=(ct == 7),
                    )
                k_ps = psmm.tile([128, 512], F32, tag="mm", name=f"kps_{r}")
                for ct in range(8):
                    nc.tensor.matmul(
                        k_ps[:], wqk_sb[:, ct, CPC:], xt[:, ct, :],
                        start=(ct == 0), stop=(ct == 7),
                    )
                # single fused bias(+scale) copies: partitions 0-63 are
                # head 0's 64 dims, 64-127 head 1's (matches row tiling)
                nc.vector.tensor_scalar(
                    qT_sb[:, rs], q_ps[:], bq_sb[:], 0.125,
                    mybir.AluOpType.add, mybir.AluOpType.mult,
                )
                nc.vector.tensor_scalar(
                    kT_sb[:, rs], k_ps[:], bk_sb[:], None,
                    mybir.AluOpType.add,
                )
                # v computed directly in [row, oc] layout: x-chunk
                # stationary, wv moving — no transposes needed
                v_ps = psmm.tile([128, 512], F32, tag="mm", name=f"vps_{r}")
                for t in range(4):
                    for ct in range(8):
                        nc.tensor.matmul(
                            v_ps[:, t * 128 : (t + 1) * 128],
                            xt[:, ct, t * 128 : (t + 1) * 128],
                            wv_sb[:, ct, :],
                            start=(ct == 0), stop=(ct == 7),
                        )
                for t in range(4):
                    slot = 4 * r + t
                    # one strided copy: [128,2,64] src (heads side by side)
                    # -> dst cols {0:64, 128:192} (stride 128)
                    nc.vector.tensor_copy(
                        out=vp_sb[:, slot, 0:256].rearrange(
                            "p (h c) -> p h c", c=128
                        )[:, :, 0:64],
                        in_=v_ps[:, t * 128 : (t + 1) * 128].rearrange(
                            "p (h c) -> p h c", c=64
                        ),
                    )

            def attn_pair(b, qc):
                with nc.named_scope(f"at{b}{qc}"):
                    _attn_pair(b, qc)

            def _attn_pair(b, qc):
                q0 = qc * 512
                nkt = 4 * qc + 4
                y_ps = [
                    psy.tile([128, 512], F32, tag="y", name=f"y_{b}_{h}_{qc}")
                    for h in range(2)
                ]

                def qk_tile(kt):
                    n = 512 - max(0, (kt - 4 * qc) * 128)
                    diag = kt >= 4 * qc
                    st = psst.tile(
                        [128, 1024], F32, tag="st", name=f"st_{b}_{qc}_{kt}"
                    )
                    ks = slice(b * T + kt * 128, b * T + (kt + 1) * 128)
                    qs = slice(b * T + q0 + 512 - n, b * T + q0 + 512)
                    # two 64-contraction matmuls on PE row-tiles (0,0) and
                    # (64,0) — they run concurrently
                    nc.tensor.matmul(
                        st[:, 0:n], kT_sb[0:64, ks], qT_sb[0:64, qs],
                        start=True, stop=True,
                    )
                    nc.tensor.matmul(
                        st[:, 512 : 512 + n], kT_sb[64:128, ks], qT_sb[64:128, qs],
                        start=True, stop=True,
                    )
                    pT = ptp.tile([128, 1024], BF, tag="pT")
                    if n == 512:
                        nc.scalar.activation(pT[:], st[:], Exp)
                    else:
                        # strided [128, 2, n] view skips the stale middle
                        nc.scalar.activation(
                            pT[:].rearrange("p (h c) -> p h c", c=512)[:, :, 0:n],
                            st[:].rearrange("p (h c) -> p h c", c=512)[:, :, 0:n],
                            Exp,
                        )
                    if diag:
                        # zero the strictly-upper triangle of the diagonal
                        # 128x128 block of each head's p (first 128 cols of
                        # each half) — replaces the -30 PE mask matmuls
                        for h in range(2):
                            nc.vector.tensor_tensor(
                                pT[:, 512 * h : 512 * h + 128],
                                pT[:, 512 * h : 512 * h + 128],
                                tril_sb[:],
                                mybir.AluOpType.mult,
                            )
                    return pT, n

                def pv_tile(kt, pT, n):
                    slot = b * NKT + kt
                    for h in range(2):
                        nc.tensor.matmul(
                            y_ps[h][:, 512 - n :],
                            vp_sb[:, slot, 128 * h : 128 * h + 128],
                            pT[:, 512 * h : 512 * h + n],
                            start=(kt == 0),
                            stop=(kt == nkt - 1),
                        )

                pend = [qk_tile(0), qk_tile(1)]
                for kt in range(nkt):
                    args = pend.pop(0)
                    if kt + 2 < nkt:
                        pend.append(qk_tile(kt + 2))
                    pv_tile(kt, *args)

                # tails: copy y & den out of PSUM in one op (frees the
                # bank), then normalize via gpsimd broadcast of 1/den
                for h in range(2):
                    ycp = tailp.tile([65, 512], F32, tag="ycp")
                    nc.vector.tensor_copy(out=ycp[:], in_=y_ps[h][0:65, :])
                    # den must land at partition base 0 before the custom-DVE
                    # reciprocal (cross-partition-base input misreads there)
                    den = tailp.tile([1, 512], F32, tag="den")
                    nc.vector.tensor_copy(out=den[:], in_=y_ps[h][64:65, :])
                    rcp = tailp.tile([1, 512], F32, tag="rcp")
                    nc.vector.reciprocal_approx_fast(rcp[:], den[:])
                    bc = tailp.tile([64, 512], F32, tag="bc")
                    nc.gpsimd.partition_broadcast(bc[:], rcp[:])
                    yT = tailp.tile([64, 512], BF, tag="yT")
                    nc.vector.tensor_tensor(
                        yT[:], ycp[0:64, :], bc[:], mybir.AluOpType.mult
                    )
                    if dbg:
                        nc.sync.dma_start(dbg_ycp[:, b * 8 + qc * 2 + h, :], ycp[:])
                    # scatter to the 2 dest cores owning this q-chunk
                    dst = a2a_in[b][:].rearrange(
                        "(d ch) q -> ch d q", ch=128
                    )[h * 64 : (h + 1) * 64, 2 * qc : 2 * qc + 2, :]
                    nc.sync.dma_start(
                        dst, yT[:].rearrange("p (s q) -> p s q", q=256)
                    )

            def fire_cc(b):
                nc.gpsimd.collective_compute(
                    "AllToAll",
                    mybir.AluOpType.bypass,
                    replica_groups=[list(range(NCORES))],
                    ins=[a2a_in[b][:].opt()],
                    outs=[a2a_out[b][:].opt()],
                )

            def yth_load(b):
                yth = ythp.tile([128, 8, 256], BF, tag="yth", name=f"yth{b}")
                nc.sync.dma_start(
                    yth[:], a2a_out[b][:].rearrange("(ct p) q -> p ct q", p=128)
                )
                return yth

            out_r = out.rearrange("(bt p) o -> p bt o", p=128)

            def proj_group(b, yth, gi):
                with nc.named_scope(f"pj{b}{gi}"):
                    _proj_group(b, yth, gi)

            def _proj_group(b, yth, gi):
                slot, oc = gi // 2, gi % 2
                ocs = slice(oc * 512, (oc + 1) * 512)
                o_ps = psmm.tile([128, 512], F32, tag="mm", name=f"o_{b}_{gi}")
                for ct in range(8):
                    nc.tensor.matmul(
                        o_ps[:],
                        yth[:, ct, slot * 128 : (slot + 1) * 128],
                        wp_sb[:, ct, ocs],
                        start=(ct == 0), stop=(ct == 7),
                    )
                # bias applied during the PSUM->SBUF copy (DVE, not PE)
                o_sb = osbp.tile([128, 512], F32, tag="osb")
                nc.vector.tensor_tensor(
                    o_sb[:], o_ps[:], bprime_bc[:, ocs], mybir.AluOpType.add
                )
                for qtr in range(2):
                    qs = slice(qtr * 256, (qtr + 1) * 256)
                    nc.sync.dma_start(
                        out_r[:, b * 2 + slot, oc * 512 + qtr * 256 : oc * 512 + (qtr + 1) * 256],
                        o_sb[:, qs],
                    )

            # ================= fused pipeline =================
            # chunks are issued one qkv r-chunk behind the one that feeds
            # them, so DVE copy-outs (qT/kT/vp) land well before the PE
            # reads them
            # qkv chunks run >=2 chunks ahead of the attn pair that
            # consumes them: the PE pulls LDWEIGHTS ahead of in-flight
            # matmuls (64-deep reorder window), so the DVE writes of
            # qT/kT/vp need real slack before the PE's weight fetch —
            # a bare semaphore is not enough on hardware.
            # proj(b-1) is issued after fire_cc(b) so its PE work hides
            # the collective; batch 3 runs its q-chunks largest-first so
            # the kernel tail ends on the smallest ACT/exp chunk.
            for b in range(3):
                qkv_r(b, 0)
                if b == 0:
                    late_consts()
                qkv_r(b, 1)
                qkv_r(b, 2)
                attn_pair(b, 0)
                qkv_r(b, 3)
                attn_pair(b, 1)
                attn_pair(b, 2)
                attn_pair(b, 3)
                fire_cc(b)
                if b >= 1:
                    yth_prev = yth_load(b - 1)
                    for gi in range(4):
                        proj_group(b - 1, yth_prev, gi)

            qkv_r(3, 0)
            qkv_r(3, 1)
            qkv_r(3, 2)
            qkv_r(3, 3)
            yth2 = yth_load(2)
            for gi in range(4):
                proj_group(2, yth2, gi)
            attn_pair(3, 3)
            attn_pair(3, 2)
            attn_pair(3, 1)
            attn_pair(3, 0)
            fire_cc(3)

            yth3 = yth_load(3)
            for gi in range(4):
                proj_group(3, yth3, gi)

            if dbg:
                nc.sync.dma_start(dbg_bpc[:], bprime_bc[:])
                for sl in range(2):
                    cs = slice(sl * 128, (sl + 1) * 128)
                    nc.sync.dma_start(dbg_ain[:, cs], a2a_in[0][sl][:])
                    nc.sync.dma_start(dbg_aout[:, cs], a2a_out[0][sl][:])
                nc.sync.dma_start(dbg_yth.rearrange("p (ct q) -> p ct q", ct=8), yth3[:])
                nc.sync.dma_start(dbg_q[:], qT_sb[:])
                nc.sync.dma_start(dbg_k[:], kT_sb[:])
                nc.sync.dma_start(
                    dbg_vp.rearrange("p (s c) -> p s c", c=256), vp_sb[:]
                )

    nc.finalize()
    return nc


def _prep_inputs(x, c_attn_w, c_attn_b, c_proj_w, c_proj_b):
    x = np.asarray(x, dtype=np.float32)
    c_attn_w = np.asarray(c_attn_w, dtype=np.float32)
    c_attn_b = np.asarray(c_attn_b, dtype=np.float32)
    c_proj_w = np.asarray(c_proj_w, dtype=np.float32)
    c_proj_b = np.asarray(c_proj_b, dtype=np.float32)

    xT = np.ascontiguousarray(x.reshape(ROWS, C).T).astype(BF16)
    wq, wk, wv_full = c_attn_w[:, :C], c_attn_w[:, C : 2 * C], c_attn_w[:, 2 * C :]
    bqf, bkf, bvf = c_attn_b[:C], c_attn_b[C : 2 * C], c_attn_b[2 * C :]
    wp_b = np.ascontiguousarray(c_proj_w).astype(BF16)
    bprime = (bvf @ c_proj_w + c_proj_b).reshape(1, C).astype(np.float32)
    # tril[k, q] = 1 where k <= q (keep), 0 above: S^T diag-block causal mask
    tril = np.triu(np.ones((128, 128), dtype=np.float32)).astype(BF16)

    in_maps = []
    for c in range(NCORES):
        cs = slice(c * CPC, (c + 1) * CPC)
        in_maps.append(
            {
                "xT": xT,
                "wqk": np.ascontiguousarray(
                    np.concatenate([wq[:, cs], wk[:, cs]], axis=1)
                ).astype(BF16),
                "wv": np.ascontiguousarray(wv_full[:, cs]).astype(BF16),
                "bq": np.ascontiguousarray(bqf[cs].reshape(CPC, 1)).astype(np.float32),
                "bk": np.ascontiguousarray(bkf[cs].reshape(CPC, 1)).astype(np.float32),
                "wp": wp_b,
                "bprime": bprime,
                "tril": tril,
            }
        )
    return in_maps


def _unshard(results):
    # core c, rows b*256 + i  ->  full row b*2048 + c*256 + i
    stk = np.stack([results[c]["out"] for c in range(NCORES)])  # [8,1024,C]
    full = stk.reshape(NCORES, B, 256, C).transpose(1, 0, 2, 3)
    return np.ascontiguousarray(full.reshape(B, T, C)).astype(np.float32)


def kernel(x, c_attn_w, c_attn_b, c_proj_w, c_proj_b):
    from concourse.bass_utils import run_bass_kernel_spmd

    if "nc" not in _CACHE:
        _CACHE["nc"] = _build()
    nc = _CACHE["nc"]

    in_maps = _prep_inputs(x, c_attn_w, c_attn_b, c_proj_w, c_proj_b)
    if "warm" not in _CACHE:
        # first execution in a process can race device-side one-time init
        # (gpsimd library residency); throw one run away
        run_bass_kernel_spmd(nc, in_maps, core_ids=list(range(NCORES)))
        _CACHE["warm"] = True
    res = run_bass_kernel_spmd(nc, in_maps, core_ids=list(range(NCORES)))
    return _unshard(res.results)


# revision 12
# speedup vs baseline: 1.0003x; 1.0003x over previous
"""Causal self-attention (B=4, T=2048, C=1024, H=16) on 8 trn2 NeuronCores.

Sharding: tensor-parallel over heads (2 heads/core) for QKV projection +
attention; output rows are resharded core-cyclically by 128-row tiles
(dest core d owns row-tiles t with t % 8 == d) so each batch gets its own
quarter-size AllToAll that serves all 8 cores symmetrically — the
collective for batch b fires as soon as batch b's attention is done and
overlaps batch b+1's compute.  Host gather interleaves the tiles back.

Key PE optimizations over the plain per-head schedule:
  * QK^T is ROW-TILED: head 0 lives on partitions 0-63 of the qT/kT
    slabs, head 1 on 64-127.  Each k-tile issues two 64-contraction
    matmuls at tile_position (0,0) and (64,0) which execute
    CONCURRENTLY on the two halves of the PE array (~2x QK throughput).
    Scores share one [128,1024] PSUM group (h0 cols 0:512, h1 512:1024)
    so a single ACT exp instruction covers both heads.
  * V is computed directly in [row, oc] layout (x-stationary matmuls)
    — no PE transposes, no vst staging copy.
  * V slots are packed [v_h0(64) | ones | v_h1(64) | ones] so the PV
    matmul of either head is a 65-wide lhsT whose output carries y in
    partitions 0-63 and the softmax denominator in partition 64 (no
    max-subtraction: logits are small).  No zero padding needed.
  * c_proj bias is applied by the DVE during the PSUM->SBUF copy
    (tensor_tensor add against a gpsimd-broadcast bias tile) instead of
    a PE matmul.
The causal mask stays a -30 additive bias via PE matmul accumulation
(identity x mask-bias) on the two diagonal-tile column blocks.
The normalizer 1/den is partition-broadcast on gpsimd; y/den are copied
out of PSUM in one [65,512] DVE op so the bank frees early.

PSUM budget (8 banks): score groups 2x2 + y accum 2x1 + shared
qkv/proj accumulator 2x1.
"""

import sys

for _p in ("/opt/trn_rl_repo",):
    if _p not in sys.path:
        sys.path.insert(0, _p)

import numpy as np
import ml_dtypes

B, T, C, H, HS = 4, 2048, 1024, 16, 64
NCORES = 8
HPC = H // NCORES            # heads per core = 2
CPC = HPC * HS               # channels per core = 128
ROWS = B * T                 # 8192
RPC = ROWS // NCORES         # rows per core = 1024
NKT = T // 128               # k-tiles per batch = 16

BF16 = ml_dtypes.bfloat16

_CACHE: dict = {}


def _apply_tile_tail_patch(tile_mod):
    """This container's walrus rejects CTRL-class instructions (Drain/NoOp)
    carrying semaphore waits. Re-emit TileContext's tail waits as individual
    EventSemaphore waits and use the sem-only barrier variant."""
    import bass_rust
    from concourse.vector_clock import ScopedClock

    if getattr(tile_mod.TileContext, "_tail_patch_applied", False):
        return

    def _drain_and_barrier(self, tick_clock, wait_clock):
        collector = self.nc.sync.nop(nofuse=True, hint="tile_tail_wait")
        wait_clock.add_sem_waits(
            collector.ins, ScopedClock({None: tick_clock.global_clock})
        )
        si = collector.ins.sync_info
        waits = list(si.on_wait) if si is not None else []
        collector.ins.sync_info = None
        for w in waits:
            assert w.wait_mode == "sem-ge-imm", w
            self.nc.sync.wait_ge(
                bass_rust.SemaphoreHandle(w.ant_name, w.id), w.wait_value
            )

        self.nc.all_engine_barrier(sem_only=True)
        assert self.sems is not None
        popped = self.nc._tile_sem_poison_stack.pop()
        assert popped is self._sem_poison
        self.nc.clear_and_free_semaphores(list(self.sems.allocated().values()))
        self.nc.all_engine_barrier(sem_only=True)

    tile_mod.TileContext._drain_and_barrier = _drain_and_barrier
    tile_mod.TileContext._tail_patch_applied = True


def _build(dbg=False):
    import concourse.bass as bass
    import concourse.bacc as bacc
    import concourse.mybir as mybir
    import concourse.tile as tile

    _apply_tile_tail_patch(tile)

    dt = mybir.dt
    F32 = dt.float32
    BF = dt.bfloat16
    Exp = mybir.ActivationFunctionType.Exp

    nc = bacc.Bacc(num_devices=NCORES)

    if dbg:
        dbg_q = nc.dram_tensor("dbg_q", [128, ROWS], BF, kind="ExternalOutput")
        dbg_k = nc.dram_tensor("dbg_k", [128, ROWS], BF, kind="ExternalOutput")
        dbg_vp = nc.dram_tensor("dbg_vp", [128, NKT * B * 256], BF, kind="ExternalOutput")
        dbg_ycp = nc.dram_tensor("dbg_ycp", [65, 32, 512], F32, kind="ExternalOutput")
        dbg_bpc = nc.dram_tensor("dbg_bpc", [128, C], F32, kind="ExternalOutput")
        dbg_ain = nc.dram_tensor("dbg_ain", [NCORES * CPC, 256], BF, kind="ExternalOutput")
        dbg_aout = nc.dram_tensor("dbg_aout", [NCORES * CPC, 256], BF, kind="ExternalOutput")
        dbg_yth = nc.dram_tensor("dbg_yth", [128, 8 * 256], BF, kind="ExternalOutput")

    xT = nc.dram_tensor("xT", [C, ROWS], BF, kind="ExternalInput")
    wqk = nc.dram_tensor("wqk", [C, 2 * CPC], BF, kind="ExternalInput")
    wv = nc.dram_tensor("wv", [C, CPC], BF, kind="ExternalInput")
    bq = nc.dram_tensor("bq", [CPC, 1], F32, kind="ExternalInput")
    bk = nc.dram_tensor("bk", [CPC, 1], F32, kind="ExternalInput")
    wp = nc.dram_tensor("wp", [C, C], BF, kind="ExternalInput")
    bprime = nc.dram_tensor("bprime", [1, C], F32, kind="ExternalInput")
    maskb = nc.dram_tensor("maskb", [128, 128], BF, kind="ExternalInput")
    ident = nc.dram_tensor("ident", [128, 128], BF, kind="ExternalInput")
    out = nc.dram_tensor("out", [RPC, C], F32, kind="ExternalOutput")

    with tile.TileContext(nc) as tc:
        with (
            tc.tile_pool(name="const", bufs=1) as constp,
            tc.tile_pool(name="big", bufs=1) as bigp,
            tc.tile_pool(name="xin", bufs=3) as xinp,
            tc.tile_pool(name="pt", bufs=3) as ptp,
            tc.tile_pool(name="tail", bufs=4) as tailp,
            tc.tile_pool(name="osb", bufs=2) as osbp,
            tc.tile_pool(name="yth", bufs=2) as ythp,
            tc.tile_pool(name="psy", bufs=2, space="PSUM") as psy,
            tc.tile_pool(name="psst", bufs=2, space="PSUM") as psst,
            tc.tile_pool(name="psmm", bufs=2, space="PSUM") as psmm,
            tc.tile_pool(name="dram", bufs=1, space="DRAM") as dramp,
        ):
            # ---- constants (only what the first r-chunk needs up front;
            # the rest is issued after the first x tile DMA so PE starts
            # ~20us earlier) ----
            # split per contraction-tile so the transfers round-robin over
            # DMA queues and the first q matmul starts ~3us in
            wqk_sb = constp.tile([128, 8, 2 * CPC], BF, tag="wqk")
            wqk_r = wqk.rearrange("(ct p) o -> p ct o", p=128)
            for ct in range(8):
                nc.sync.dma_start(wqk_sb[:, ct, :], wqk_r[:, ct, :])
            bq_sb = constp.tile([CPC, 1], F32, tag="bq")
            nc.sync.dma_start(bq_sb[:], bq[:])
            bk_sb = constp.tile([CPC, 1], F32, tag="bk")
            nc.sync.dma_start(bk_sb[:], bk[:])
            wv_sb = constp.tile([128, 8, CPC], BF, tag="wv")
            wv_r = wv.rearrange("(ct p) o -> p ct o", p=128)
            for ct in range(8):
                nc.sync.dma_start(wv_sb[:, ct, :], wv_r[:, ct, :])
            wp_sb = constp.tile([128, 8, C], BF, tag="wp")
            bprime_sb = constp.tile([1, C], F32, tag="bprime")
            bprime_bc = constp.tile([128, C], F32, tag="bprime_bc")
            maskb_sb = constp.tile([128, 128], BF, tag="maskb")
            nc.sync.dma_start(maskb_sb[:], maskb[:])
            ident_sb = constp.tile([128, 128], BF, tag="ident")
            nc.sync.dma_start(ident_sb[:], ident[:])

            # dummy broadcast: forces the gpsimd Q7 library resident before
            # the first real per-chunk broadcast (~15us in) — the library
            # load otherwise races it on the first execution in a process
            warm_src = constp.tile([1, 512], F32, tag="wsrc")
            nc.vector.memset(warm_src[:], 1.0)
            warm_dst = constp.tile([64, 512], F32, tag="wdst")
            nc.gpsimd.partition_broadcast(warm_dst[:], warm_src[:])

            def late_consts():
                nc.sync.dma_start(bprime_sb[:], bprime[:])
                nc.gpsimd.partition_broadcast(bprime_bc[:], bprime_sb[:])
                nc.sync.dma_start(
                    wp_sb[:], wp.rearrange("(ct p) o -> p ct o", p=128)
                )

            # ---- persistent intermediates ----
            # qT/kT: [128, ROWS] — head 0 channels on partitions 0-63,
            # head 1 on 64-127 (the row-tiling layout).
            qT_sb = bigp.tile([128, ROWS], BF, tag="qT")
            kT_sb = bigp.tile([128, ROWS], BF, tag="kT")
            # v' per global k-tile: [128 rows, 64 slots, 2*128]; per
            # head half: [v 64 | ones | 63 zeros] (the 65-col lhsT read
            # produced garbage on hw; 128-col weights are the safe path).
            vp_sb = bigp.tile([128, NKT * B, 256], BF, tag="vp")
            vpr = vp_sb[:].rearrange("p s (h c) -> p s h c", c=128)
            nc.vector.memset(vpr[:, :, :, 64:65], 1.0)
            nc.vector.memset(vpr[:, :, :, 65:128], 0.0)

            # per-batch AllToAll buffers: [8 dests x 128ch, 256 q]; dest
            # core d owns the contiguous 256-row block d of each batch, so
            # every tail's scatter lands in 512B-contiguous runs (the
            # 128-row-cyclic layout halved the DMA run length)
            a2a_in = [
                dramp.tile([NCORES * CPC, 256], BF, name=f"a2a_in{b}")
                for b in range(B)
            ]
            a2a_out = [
                dramp.tile([NCORES * CPC, 256], BF, name=f"a2a_out{b}")
                for b in range(B)
            ]

            xT_r = xT.rearrange("(ct p) r -> p ct r", p=128)

            def qkv_r(b, rb):
                with nc.named_scope(f"qkv{b}{rb}"):
                    _qkv_r(b, rb)

            def _qkv_r(b, rb):
                r = b * 4 + rb
                rs = slice(r * 512, (r + 1) * 512)
                xt = xinp.tile([128, 8, 512], BF, tag="xt")
                for cth in range(4):
                    nc.sync.dma_start(
                        xt[:, 2 * cth : 2 * cth + 2, :],
                        xT_r[:, 2 * cth : 2 * cth + 2, rs],
                    )

                q_ps = psmm.tile([128, 512], F32, tag="mm", name=f"qps_{r}")
                for ct in range(8):
                    nc.tensor.matmul(
                        q_ps[:], wqk_sb[:, ct, 0:CPC], xt[:, ct, :],
                        start=(ct == 0), stop=(ct == 7),
                    )
                k_ps = psmm.tile([128, 512], F32, tag="mm", name=f"kps_{r}")
                for ct in range(8):
                    nc.tensor.matmul(
                        k_ps[:], wqk_sb[:, ct, CPC:], xt[:, ct, :],
                        start=(ct == 0), stop=(ct == 7),
                    )
                # single fused bias(+scale) copies: partitions 0-63 are
                # head 0's 64 dims, 64-127 head 1's (matches row tiling)
                nc.vector.tensor_scalar(
                    qT_sb[:, rs], q_ps[:], bq_sb[:], 0.125,
                    mybir.AluOpType.add, mybir.AluOpType.mult,
                )
                nc.vector.tensor_scalar(
                    kT_sb[:, rs], k_ps[:], bk_sb[:], None,
                    mybir.AluOpType.add,
                )
                # v computed directly in [row, oc] layout: x-chunk
                # stationary, wv moving — no transposes needed
                v_ps = psmm.tile([128, 512], F32, tag="mm", name=f"vps_{r}")
                for t in range(4):
                    for ct in range(8):
                        nc.tensor.matmul(
                            v_ps[:, t * 128 : (t + 1) * 128],
                            xt[:, ct, t * 128 : (t + 1) * 128],
                            wv_sb[:, ct, :],
                            start=(ct == 0), stop=(ct == 7),
                        )
                for t in range(4):
                    slot = 4 * r + t
                    # one strided copy: [128,2,64] src (heads side by side)
                    # -> dst cols {0:64, 128:192} (stride 128)
                    nc.vector.tensor_copy(
                        out=vp_sb[:, slot, 0:256].rearrange(
                            "p (h c) -> p h c", c=128
                        )[:, :, 0:64],
                        in_=v_ps[:, t * 128 : (t + 1) * 128].rearrange(
                            "p (h c) -> p h c", c=64
                        ),
                    )

            def attn_pair(b, qc):
                with nc.named_scope(f"at{b}{qc}"):
                    _attn_pair(b, qc)

            def _attn_pair(b, qc):
                q0 = qc * 512
                nkt = 4 * qc + 4
                y_ps = [
                    psy.tile([128, 512], F32, tag="y", name=f"y_{b}_{h}_{qc}")
                    for h in range(2)
                ]

                def qk_tile(kt):
                    n = 512 - max(0, (kt - 4 * qc) * 128)
                    diag = kt >= 4 * qc
                    st = psst.tile(
                        [128, 1024], F32, tag="st", name=f"st_{b}_{qc}_{kt}"
                    )
                    ks = slice(b * T + kt * 128, b * T + (kt + 1) * 128)
                    qs = slice(b * T + q0 + 512 - n, b * T + q0 + 512)
                    # two 64-contraction matmuls on PE row-tiles (0,0) and
                    # (64,0) — they run concurrently
                    nc.tensor.matmul(
                        st[:, 0:n], kT_sb[0:64, ks], qT_sb[0:64, qs],
                        start=True, stop=not diag,
                    )
                    nc.tensor.matmul(
                        st[:, 512 : 512 + n], kT_sb[64:128, ks], qT_sb[64:128, qs],
                        start=True, stop=not diag,
                    )
                    if diag:
                        nc.tensor.matmul(
                            st[:, 0:128], ident_sb[:], maskb_sb[:],
                            start=False, stop=True,
                        )
                        nc.tensor.matmul(
                            st[:, 512:640], ident_sb[:], maskb_sb[:],
                            start=False, stop=True,
                        )
                    pT = ptp.tile([128, 1024], BF, tag="pT")
                    if n == 512:
                        nc.scalar.activation(pT[:], st[:], Exp)
                    else:
                        # strided [128, 2, n] view skips the stale middle
                        nc.scalar.activation(
                            pT[:].rearrange("p (h c) -> p h c", c=512)[:, :, 0:n],
                            st[:].rearrange("p (h c) -> p h c", c=512)[:, :, 0:n],
                            Exp,
                        )
                    return pT, n

                def pv_tile(kt, pT, n):
                    slot = b * NKT + kt
                    for h in range(2):
                        nc.tensor.matmul(
                            y_ps[h][:, 512 - n :],
                            vp_sb[:, slot, 128 * h : 128 * h + 128],
                            pT[:, 512 * h : 512 * h + n],
                            start=(kt == 0),
                            stop=(kt == nkt - 1),
                        )

                pend = [qk_tile(0), qk_tile(1)]
                for kt in range(nkt):
                    args = pend.pop(0)
                    if kt + 2 < nkt:
                        pend.append(qk_tile(kt + 2))
                    pv_tile(kt, *args)

                # tails: copy y & den out of PSUM in one op (frees the
                # bank), then normalize via gpsimd broadcast of 1/den
                for h in range(2):
                    ycp = tailp.tile([65, 512], F32, tag="ycp")
                    nc.vector.tensor_copy(out=ycp[:], in_=y_ps[h][0:65, :])
                    # den must land at partition base 0 before the custom-DVE
                    # reciprocal (cross-partition-base input misreads there)
                    den = tailp.tile([1, 512], F32, tag="den")
                    nc.vector.tensor_copy(out=den[:], in_=y_ps[h][64:65, :])
                    rcp = tailp.tile([1, 512], F32, tag="rcp")
                    nc.vector.reciprocal_approx_fast(rcp[:], den[:])
                    bc = tailp.tile([64, 512], F32, tag="bc")
                    nc.gpsimd.partition_broadcast(bc[:], rcp[:])
                    yT = tailp.tile([64, 512], BF, tag="yT")
                    nc.vector.tensor_tensor(
                        yT[:], ycp[0:64, :], bc[:], mybir.AluOpType.mult
                    )
                    if dbg:
                        nc.sync.dma_start(dbg_ycp[:, b * 8 + qc * 2 + h, :], ycp[:])
                    # scatter to the 2 dest cores owning this q-chunk
                    dst = a2a_in[b][:].rearrange(
                        "(d ch) q -> ch d q", ch=128
                    )[h * 64 : (h + 1) * 64, 2 * qc : 2 * qc + 2, :]
                    nc.sync.dma_start(
                        dst, yT[:].rearrange("p (s q) -> p s q", q=256)
                    )

            def fire_cc(b):
                nc.gpsimd.collective_compute(
                    "AllToAll",
                    mybir.AluOpType.bypass,
                    replica_groups=[list(range(NCORES))],
                    ins=[a2a_in[b][:].opt()],
                    outs=[a2a_out[b][:].opt()],
                )

            def yth_load(b):
                yth = ythp.tile([128, 8, 256], BF, tag="yth", name=f"yth{b}")
                nc.sync.dma_start(
                    yth[:], a2a_out[b][:].rearrange("(ct p) q -> p ct q", p=128)
                )
                return yth

            out_r = out.rearrange("(bt p) o -> p bt o", p=128)

            def proj_group(b, yth, gi):
                with nc.named_scope(f"pj{b}{gi}"):
                    _proj_group(b, yth, gi)

            def _proj_group(b, yth, gi):
                slot, oc = gi // 2, gi % 2
                ocs = slice(oc * 512, (oc + 1) * 512)
                o_ps = psmm.tile([128, 512], F32, tag="mm", name=f"o_{b}_{gi}")
                for ct in range(8):
                    nc.tensor.matmul(
                        o_ps[:],
                        yth[:, ct, slot * 128 : (slot + 1) * 128],
                        wp_sb[:, ct, ocs],
                        start=(ct == 0), stop=(ct == 7),
                    )
                # bias applied during the PSUM->SBUF copy (DVE, not PE)
                o_sb = osbp.tile([128, 512], F32, tag="osb")
                nc.vector.tensor_tensor(
                    o_sb[:], o_ps[:], bprime_bc[:, ocs], mybir.AluOpType.add
                )
                for qtr in range(2):
                    qs = slice(qtr * 256, (qtr + 1) * 256)
                    nc.sync.dma_start(
                        out_r[:, b * 2 + slot, oc * 512 + qtr * 256 : oc * 512 + (qtr + 1) * 256],
                        o_sb[:, qs],
                    )

            # ================= fused pipeline =================
            # chunks are issued one qkv r-chunk behind the one that feeds
            # them, so DVE copy-outs (qT/kT/vp) land well before the PE
            # reads them
            # qkv chunks run >=2 chunks ahead of the attn pair that
            # consumes them: the PE pulls LDWEIGHTS ahead of in-flight
            # matmuls (64-deep reorder window), so the DVE writes of
            # qT/kT/vp need real slack before the PE's weight fetch —
            # a bare semaphore is not enough on hardware.
            # proj(b-1) is issued after fire_cc(b) so its PE work hides
            # the collective; batch 3 runs its q-chunks largest-first so
            # the kernel tail ends on the smallest ACT/exp chunk.
            for b in range(3):
                qkv_r(b, 0)
                if b == 0:
                    late_consts()
                qkv_r(b, 1)
                qkv_r(b, 2)
                attn_pair(b, 0)
                qkv_r(b, 3)
                attn_pair(b, 1)
                attn_pair(b, 2)
                attn_pair(b, 3)
                fire_cc(b)
                if b >= 1:
                    yth_prev = yth_load(b - 1)
                    for gi in range(4):
                        proj_group(b - 1, yth_prev, gi)

            qkv_r(3, 0)
            qkv_r(3, 1)
            qkv_r(3, 2)
            qkv_r(3, 3)
            yth2 = yth_load(2)
            for gi in range(4):
                proj_group(2, yth2, gi)
            attn_pair(3, 3)
            attn_pair(3, 2)
            attn_pair(3, 1)
            attn_pair(3, 0)
            fire_cc(3)

            yth3 = yth_load(3)
            for gi in range(4):
                proj_group(3, yth3, gi)

            if dbg:
                nc.sync.dma_start(dbg_bpc[:], bprime_bc[:])
                for sl in range(2):
                    cs = slice(sl * 128, (sl + 1) * 128)
                    nc.sync.dma_start(dbg_ain[:, cs], a2a_in[0][sl][:])
                    nc.sync.dma_start(dbg_aout[:, cs], a2a_out[0][sl][:])
                nc.sync.dma_start(dbg_yth.rearrange("p (ct q) -> p ct q", ct=8), yth3[:])
                nc.sync.dma_start(dbg_q[:], qT_sb[:])
                nc.sync.dma_start(dbg_k[:], kT_sb[:])
                nc.sync.dma_start(
                    dbg_vp.rearrange("p (s c) -> p s c", c=256), vp_sb[:]
                )

    nc.finalize()
    return nc


def _prep_inputs(x, c_attn_w, c_attn_b, c_proj_w, c_proj_b):
    x = np.asarray(x, dtype=np.float32)
    c_attn_w = np.asarray(c_attn_w, dtype=np.float32)
    c_attn_b = np.asarray(c_attn_b, dtype=np.float32)
    c_proj_w = np.asarray(c_proj_w, dtype=np.float32)
    c_proj_b = np.asarray(c_proj_b, dtype=np.float32)

    xT = np.ascontiguousarray(x.reshape(ROWS, C).T).astype(BF16)
    wq, wk, wv_full = c_attn_w[:, :C], c_attn_w[:, C : 2 * C], c_attn_w[:, 2 * C :]
    bqf, bkf, bvf = c_attn_b[:C], c_attn_b[C : 2 * C], c_attn_b[2 * C :]
    wp_b = np.ascontiguousarray(c_proj_w).astype(BF16)
    bprime = (bvf @ c_proj_w + c_proj_b).reshape(1, C).astype(np.float32)
    maskb = np.tril(np.full((128, 128), -30.0, dtype=np.float32), -1).astype(BF16)
    ident = np.eye(128, dtype=np.float32).astype(BF16)

    in_maps = []
    for c in range(NCORES):
        cs = slice(c * CPC, (c + 1) * CPC)
        in_maps.append(
            {
                "xT": xT,
                "wqk": np.ascontiguousarray(
                    np.concatenate([wq[:, cs], wk[:, cs]], axis=1)
                ).astype(BF16),
                "wv": np.ascontiguousarray(wv_full[:, cs]).astype(BF16),
                "bq": np.ascontiguousarray(bqf[cs].reshape(CPC, 1)).astype(np.float32),
                "bk": np.ascontiguousarray(bkf[cs].reshape(CPC, 1)).astype(np.float32),
                "wp": wp_b,
                "bprime": bprime,
                "maskb": maskb,
                "ident": ident,
            }
        )
    return in_maps


def _unshard(results):
    # core c, rows b*256 + i  ->  full row b*2048 + c*256 + i
    stk = np.stack([results[c]["out"] for c in range(NCORES)])  # [8,1024,C]
    full = stk.reshape(NCORES, B, 256, C).transpose(1, 0, 2, 3)
    return np.ascontiguousarray(full.reshape(B, T, C)).astype(np.float32)


def kernel(x, c_attn_w, c_attn_b, c_proj_w, c_proj_b):
    from concourse.bass_utils import run_bass_kernel_spmd

    if "nc" not in _CACHE:
        _CACHE["nc"] = _build()
    nc = _CACHE["nc"]

    in_maps = _prep_inputs(x, c_attn_w, c_attn_b, c_proj_w, c_proj_b)
    if "warm" not in _CACHE:
        # first execution in a process can race device-side one-time init
        # (gpsimd library residency); throw one run away
        run_bass_kernel_spmd(nc, in_maps, core_ids=list(range(NCORES)))
        _CACHE["warm"] = True
    res = run_bass_kernel_spmd(nc, in_maps, core_ids=list(range(NCORES)))
    return _unshard(res.results)


# revision 13
# speedup vs baseline: 1.0258x; 1.0255x over previous
"""Causal self-attention (B=4, T=2048, C=1024, H=16) on 8 trn2 NeuronCores.

Sharding: tensor-parallel over heads (2 heads/core) for QKV projection +
attention; output rows are resharded core-cyclically by 128-row tiles
(dest core d owns row-tiles t with t % 8 == d) so each batch gets its own
quarter-size AllToAll that serves all 8 cores symmetrically — the
collective for batch b fires as soon as batch b's attention is done and
overlaps batch b+1's compute.  Host gather interleaves the tiles back.

Key PE optimizations over the plain per-head schedule:
  * QK^T is ROW-TILED: head 0 lives on partitions 0-63 of the qT/kT
    slabs, head 1 on 64-127.  Each k-tile issues two 64-contraction
    matmuls at tile_position (0,0) and (64,0) which execute
    CONCURRENTLY on the two halves of the PE array (~2x QK throughput).
    Scores share one [128,1024] PSUM group (h0 cols 0:512, h1 512:1024)
    so a single ACT exp instruction covers both heads.
  * V is computed directly in [row, oc] layout (x-stationary matmuls)
    — no PE transposes, no vst staging copy.
  * V slots are packed [v_h0(64) | ones | v_h1(64) | ones] so the PV
    matmul of either head is a 65-wide lhsT whose output carries y in
    partitions 0-63 and the softmax denominator in partition 64 (no
    max-subtraction: logits are small).  No zero padding needed.
  * c_proj bias is applied by the DVE during the PSUM->SBUF copy
    (tensor_tensor add against a gpsimd-broadcast bias tile) instead of
    a PE matmul.
The causal mask stays a -30 additive bias via PE matmul accumulation
(identity x mask-bias) on the two diagonal-tile column blocks.
The normalizer 1/den is partition-broadcast on gpsimd; y/den are copied
out of PSUM in one [65,512] DVE op so the bank frees early.

PSUM budget (8 banks): score groups 2x2 + y accum 2x1 + shared
qkv/proj accumulator 2x1.
"""

import sys

for _p in ("/opt/trn_rl_repo",):
    if _p not in sys.path:
        sys.path.insert(0, _p)

import numpy as np
import ml_dtypes

B, T, C, H, HS = 4, 2048, 1024, 16, 64
NCORES = 8
HPC = H // NCORES            # heads per core = 2
CPC = HPC * HS               # channels per core = 128
ROWS = B * T                 # 8192
RPC = ROWS // NCORES         # rows per core = 1024
NKT = T // 128               # k-tiles per batch = 16

BF16 = ml_dtypes.bfloat16

_CACHE: dict = {}


def _apply_tile_tail_patch(tile_mod):
    """This container's walrus rejects CTRL-class instructions (Drain/NoOp)
    carrying semaphore waits. Re-emit TileContext's tail waits as individual
    EventSemaphore waits and use the sem-only barrier variant."""
    import bass_rust
    from concourse.vector_clock import ScopedClock

    if getattr(tile_mod.TileContext, "_tail_patch_applied", False):
        return

    def _drain_and_barrier(self, tick_clock, wait_clock):
        collector = self.nc.sync.nop(nofuse=True, hint="tile_tail_wait")
        wait_clock.add_sem_waits(
            collector.ins, ScopedClock({None: tick_clock.global_clock})
        )
        si = collector.ins.sync_info
        waits = list(si.on_wait) if si is not None else []
        collector.ins.sync_info = None
        for w in waits:
            assert w.wait_mode == "sem-ge-imm", w
            self.nc.sync.wait_ge(
                bass_rust.SemaphoreHandle(w.ant_name, w.id), w.wait_value
            )

        self.nc.all_engine_barrier(sem_only=True)
        assert self.sems is not None
        popped = self.nc._tile_sem_poison_stack.pop()
        assert popped is self._sem_poison
        self.nc.clear_and_free_semaphores(list(self.sems.allocated().values()))
        self.nc.all_engine_barrier(sem_only=True)

    tile_mod.TileContext._drain_and_barrier = _drain_and_barrier
    tile_mod.TileContext._tail_patch_applied = True


def _build(dbg=False):
    import concourse.bass as bass
    import concourse.bacc as bacc
    import concourse.mybir as mybir
    import concourse.tile as tile

    _apply_tile_tail_patch(tile)

    dt = mybir.dt
    F32 = dt.float32
    BF = dt.bfloat16
    Exp = mybir.ActivationFunctionType.Exp

    nc = bacc.Bacc(num_devices=NCORES)

    if dbg:
        dbg_q = nc.dram_tensor("dbg_q", [128, ROWS], BF, kind="ExternalOutput")
        dbg_k = nc.dram_tensor("dbg_k", [128, ROWS], BF, kind="ExternalOutput")
        dbg_vp = nc.dram_tensor("dbg_vp", [128, NKT * B * 256], BF, kind="ExternalOutput")
        dbg_ycp = nc.dram_tensor("dbg_ycp", [65, 32, 512], F32, kind="ExternalOutput")
        dbg_bpc = nc.dram_tensor("dbg_bpc", [128, C], F32, kind="ExternalOutput")
        dbg_ain = nc.dram_tensor("dbg_ain", [NCORES * CPC, 256], BF, kind="ExternalOutput")
        dbg_aout = nc.dram_tensor("dbg_aout", [NCORES * CPC, 256], BF, kind="ExternalOutput")
        dbg_yth = nc.dram_tensor("dbg_yth", [128, 8 * 256], BF, kind="ExternalOutput")

    xT = nc.dram_tensor("xT", [C, ROWS], BF, kind="ExternalInput")
    wqk = nc.dram_tensor("wqk", [C, 2 * CPC], BF, kind="ExternalInput")
    wv = nc.dram_tensor("wv", [C, CPC], BF, kind="ExternalInput")
    bq = nc.dram_tensor("bq", [CPC, 1], F32, kind="ExternalInput")
    bk = nc.dram_tensor("bk", [CPC, 1], F32, kind="ExternalInput")
    wp = nc.dram_tensor("wp", [C, C], BF, kind="ExternalInput")
    bprime = nc.dram_tensor("bprime", [1, C], F32, kind="ExternalInput")
    maskb = nc.dram_tensor("maskb", [128, 128], BF, kind="ExternalInput")
    ident = nc.dram_tensor("ident", [128, 128], BF, kind="ExternalInput")
    out = nc.dram_tensor("out", [RPC, C], F32, kind="ExternalOutput")

    with tile.TileContext(nc) as tc:
        with (
            tc.tile_pool(name="const", bufs=1) as constp,
            tc.tile_pool(name="big", bufs=1) as bigp,
            tc.tile_pool(name="xin", bufs=3) as xinp,
            tc.tile_pool(name="pt", bufs=3) as ptp,
            tc.tile_pool(name="tail", bufs=4) as tailp,
            tc.tile_pool(name="osb", bufs=2) as osbp,
            tc.tile_pool(name="yth", bufs=2) as ythp,
            tc.tile_pool(name="psy", bufs=2, space="PSUM") as psy,
            tc.tile_pool(name="psst", bufs=2, space="PSUM") as psst,
            tc.tile_pool(name="psmm", bufs=2, space="PSUM") as psmm,
            tc.tile_pool(name="dram0", bufs=1, space="DRAM") as dramp0,
            tc.tile_pool(name="dram1", bufs=1, space="DRAM") as dramp1,
            tc.tile_pool(name="dram2", bufs=1, space="DRAM") as dramp2,
            tc.tile_pool(name="dram3", bufs=1, space="DRAM") as dramp3,
        ):
            dramps = [dramp0, dramp1, dramp2, dramp3]
            # ---- constants (only what the first r-chunk needs up front;
            # the rest is issued after the first x tile DMA so PE starts
            # ~20us earlier) ----
            # split per contraction-tile so the transfers round-robin over
            # DMA queues and the first q matmul starts ~3us in
            wqk_sb = constp.tile([128, 8, 2 * CPC], BF, tag="wqk")
            wqk_r = wqk.rearrange("(ct p) o -> p ct o", p=128)
            for ct in range(8):
                nc.sync.dma_start(wqk_sb[:, ct, :], wqk_r[:, ct, :])
            bq_sb = constp.tile([CPC, 1], F32, tag="bq")
            nc.sync.dma_start(bq_sb[:], bq[:])
            bk_sb = constp.tile([CPC, 1], F32, tag="bk")
            nc.sync.dma_start(bk_sb[:], bk[:])
            wv_sb = constp.tile([128, 8, CPC], BF, tag="wv")
            wv_r = wv.rearrange("(ct p) o -> p ct o", p=128)
            for ct in range(8):
                nc.sync.dma_start(wv_sb[:, ct, :], wv_r[:, ct, :])
            wp_sb = constp.tile([128, 8, C], BF, tag="wp")
            bprime_sb = constp.tile([1, C], F32, tag="bprime")
            bprime_bc = constp.tile([128, C], F32, tag="bprime_bc")
            maskb_sb = constp.tile([128, 128], BF, tag="maskb")
            nc.sync.dma_start(maskb_sb[:], maskb[:])
            ident_sb = constp.tile([128, 128], BF, tag="ident")
            nc.sync.dma_start(ident_sb[:], ident[:])

            # dummy broadcast: forces the gpsimd Q7 library resident before
            # the first real per-chunk broadcast (~15us in) — the library
            # load otherwise races it on the first execution in a process
            warm_src = constp.tile([1, 512], F32, tag="wsrc")
            nc.vector.memset(warm_src[:], 1.0)
            warm_dst = constp.tile([64, 512], F32, tag="wdst")
            nc.gpsimd.partition_broadcast(warm_dst[:], warm_src[:])

            def late_consts():
                nc.sync.dma_start(bprime_sb[:], bprime[:])
                nc.gpsimd.partition_broadcast(bprime_bc[:], bprime_sb[:])
                nc.sync.dma_start(
                    wp_sb[:], wp.rearrange("(ct p) o -> p ct o", p=128)
                )

            # ---- persistent intermediates ----
            # qT/kT: [128, ROWS] — head 0 channels on partitions 0-63,
            # head 1 on 64-127 (the row-tiling layout).
            qT_sb = bigp.tile([128, ROWS], BF, tag="qT")
            kT_sb = bigp.tile([128, ROWS], BF, tag="kT")
            # v' per global k-tile: [128 rows, 64 slots, 2*128]; per
            # head half: [v 64 | ones | 63 zeros] (the 65-col lhsT read
            # produced garbage on hw; 128-col weights are the safe path).
            vp_sb = bigp.tile([128, NKT * B, 256], BF, tag="vp")
            vpr = vp_sb[:].rearrange("p s (h c) -> p s h c", c=128)
            nc.vector.memset(vpr[:, :, :, 64:65], 1.0)
            nc.vector.memset(vpr[:, :, :, 65:128], 0.0)

            # per-batch AllToAll buffers: [8 dests x 128ch, 256 q]; dest
            # core d owns the contiguous 256-row block d of each batch, so
            # every tail's scatter lands in 512B-contiguous runs (the
            # 128-row-cyclic layout halved the DMA run length)
            # one DRAM pool per batch: pool-level dependency tracking
            # otherwise serializes batch b+1's scatter against the
            # collective read of batch b (false WAR)
            a2a_in = [
                dramps[b].tile([NCORES * CPC, 256], BF, name=f"a2a_in{b}")
                for b in range(B)
            ]
            a2a_out = [
                dramps[b].tile([NCORES * CPC, 256], BF, name=f"a2a_out{b}")
                for b in range(B)
            ]

            xT_r = xT.rearrange("(ct p) r -> p ct r", p=128)

            def qkv_r(b, rb):
                with nc.named_scope(f"qkv{b}{rb}"):
                    _qkv_r(b, rb)

            def _qkv_r(b, rb):
                r = b * 4 + rb
                rs = slice(r * 512, (r + 1) * 512)
                xt = xinp.tile([128, 8, 512], BF, tag="xt")
                for cth in range(4):
                    nc.sync.dma_start(
                        xt[:, 2 * cth : 2 * cth + 2, :],
                        xT_r[:, 2 * cth : 2 * cth + 2, rs],
                    )

                q_ps = psmm.tile([128, 512], F32, tag="mm", name=f"qps_{r}")
                for ct in range(8):
                    nc.tensor.matmul(
                        q_ps[:], wqk_sb[:, ct, 0:CPC], xt[:, ct, :],
                        start=(ct == 0), stop=(ct == 7),
                    )
                k_ps = psmm.tile([128, 512], F32, tag="mm", name=f"kps_{r}")
                for ct in range(8):
                    nc.tensor.matmul(
                        k_ps[:], wqk_sb[:, ct, CPC:], xt[:, ct, :],
                        start=(ct == 0), stop=(ct == 7),
                    )
                # single fused bias(+scale) copies: partitions 0-63 are
                # head 0's 64 dims, 64-127 head 1's (matches row tiling)
                nc.vector.tensor_scalar(
                    qT_sb[:, rs], q_ps[:], bq_sb[:], 0.125,
                    mybir.AluOpType.add, mybir.AluOpType.mult,
                )
                nc.vector.tensor_scalar(
                    kT_sb[:, rs], k_ps[:], bk_sb[:], None,
                    mybir.AluOpType.add,
                )
                # v computed directly in [row, oc] layout: x-chunk
                # stationary, wv moving — no transposes needed
                v_ps = psmm.tile([128, 512], F32, tag="mm", name=f"vps_{r}")
                for t in range(4):
                    for ct in range(8):
                        nc.tensor.matmul(
                            v_ps[:, t * 128 : (t + 1) * 128],
                            xt[:, ct, t * 128 : (t + 1) * 128],
                            wv_sb[:, ct, :],
                            start=(ct == 0), stop=(ct == 7),
                        )
                for t in range(4):
                    slot = 4 * r + t
                    # one strided copy: [128,2,64] src (heads side by side)
                    # -> dst cols {0:64, 128:192} (stride 128)
                    nc.vector.tensor_copy(
                        out=vp_sb[:, slot, 0:256].rearrange(
                            "p (h c) -> p h c", c=128
                        )[:, :, 0:64],
                        in_=v_ps[:, t * 128 : (t + 1) * 128].rearrange(
                            "p (h c) -> p h c", c=64
                        ),
                    )

            def attn_pair(b, qc):
                with nc.named_scope(f"at{b}{qc}"):
                    _attn_pair(b, qc)

            def _attn_pair(b, qc):
                q0 = qc * 512
                nkt = 4 * qc + 4
                y_ps = [
                    psy.tile([128, 512], F32, tag="y", name=f"y_{b}_{h}_{qc}")
                    for h in range(2)
                ]

                def qk_tile(kt):
                    n = 512 - max(0, (kt - 4 * qc) * 128)
                    diag = kt >= 4 * qc
                    st = psst.tile(
                        [128, 1024], F32, tag="st", name=f"st_{b}_{qc}_{kt}"
                    )
                    ks = slice(b * T + kt * 128, b * T + (kt + 1) * 128)
                    qs = slice(b * T + q0 + 512 - n, b * T + q0 + 512)
                    # two 64-contraction matmuls on PE row-tiles (0,0) and
                    # (64,0) — they run concurrently
                    nc.tensor.matmul(
                        st[:, 0:n], kT_sb[0:64, ks], qT_sb[0:64, qs],
                        start=True, stop=not diag,
                    )
                    nc.tensor.matmul(
                        st[:, 512 : 512 + n], kT_sb[64:128, ks], qT_sb[64:128, qs],
                        start=True, stop=not diag,
                    )
                    if diag:
                        nc.tensor.matmul(
                            st[:, 0:128], ident_sb[:], maskb_sb[:],
                            start=False, stop=True,
                        )
                        nc.tensor.matmul(
                            st[:, 512:640], ident_sb[:], maskb_sb[:],
                            start=False, stop=True,
                        )
                    pT = ptp.tile([128, 1024], BF, tag="pT")
                    if n == 512:
                        nc.scalar.activation(pT[:], st[:], Exp)
                    else:
                        # strided [128, 2, n] view skips the stale middle
                        nc.scalar.activation(
                            pT[:].rearrange("p (h c) -> p h c", c=512)[:, :, 0:n],
                            st[:].rearrange("p (h c) -> p h c", c=512)[:, :, 0:n],
                            Exp,
                        )
                    return pT, n

                def pv_tile(kt, pT, n):
                    slot = b * NKT + kt
                    for h in range(2):
                        nc.tensor.matmul(
                            y_ps[h][:, 512 - n :],
                            vp_sb[:, slot, 128 * h : 128 * h + 128],
                            pT[:, 512 * h : 512 * h + n],
                            start=(kt == 0),
                            stop=(kt == nkt - 1),
                        )

                pend = [qk_tile(0), qk_tile(1)]
                for kt in range(nkt):
                    args = pend.pop(0)
                    if kt + 2 < nkt:
                        pend.append(qk_tile(kt + 2))
                    pv_tile(kt, *args)

                # tails: copy y & den out of PSUM in one op (frees the
                # bank), then normalize via gpsimd broadcast of 1/den
                for h in range(2):
                    ycp = tailp.tile([65, 512], F32, tag="ycp")
                    nc.vector.tensor_copy(out=ycp[:], in_=y_ps[h][0:65, :])
                    # den must land at partition base 0 before the custom-DVE
                    # reciprocal (cross-partition-base input misreads there)
                    den = tailp.tile([1, 512], F32, tag="den")
                    nc.vector.tensor_copy(out=den[:], in_=y_ps[h][64:65, :])
                    rcp = tailp.tile([1, 512], F32, tag="rcp")
                    nc.vector.reciprocal_approx_fast(rcp[:], den[:])
                    bc = tailp.tile([64, 512], F32, tag="bc")
                    nc.gpsimd.partition_broadcast(bc[:], rcp[:])
                    yT = tailp.tile([64, 512], BF, tag="yT")
                    nc.vector.tensor_tensor(
                        yT[:], ycp[0:64, :], bc[:], mybir.AluOpType.mult
                    )
                    if dbg:
                        nc.sync.dma_start(dbg_ycp[:, b * 8 + qc * 2 + h, :], ycp[:])
                    # scatter to the 2 dest cores owning this q-chunk
                    dst = a2a_in[b][:].rearrange(
                        "(d ch) q -> ch d q", ch=128
                    )[h * 64 : (h + 1) * 64, 2 * qc : 2 * qc + 2, :]
                    nc.sync.dma_start(
                        dst, yT[:].rearrange("p (s q) -> p s q", q=256)
                    )

            def fire_cc(b):
                nc.gpsimd.collective_compute(
                    "AllToAll",
                    mybir.AluOpType.bypass,
                    replica_groups=[list(range(NCORES))],
                    ins=[a2a_in[b][:].opt()],
                    outs=[a2a_out[b][:].opt()],
                )

            def yth_load(b):
                yth = ythp.tile([128, 8, 256], BF, tag="yth", name=f"yth{b}")
                nc.sync.dma_start(
                    yth[:], a2a_out[b][:].rearrange("(ct p) q -> p ct q", p=128)
                )
                return yth

            out_r = out.rearrange("(bt p) o -> p bt o", p=128)

            def proj_group(b, yth, gi):
                with nc.named_scope(f"pj{b}{gi}"):
                    _proj_group(b, yth, gi)

            def _proj_group(b, yth, gi):
                slot, oc = gi // 2, gi % 2
                ocs = slice(oc * 512, (oc + 1) * 512)
                o_ps = psmm.tile([128, 512], F32, tag="mm", name=f"o_{b}_{gi}")
                for ct in range(8):
                    nc.tensor.matmul(
                        o_ps[:],
                        yth[:, ct, slot * 128 : (slot + 1) * 128],
                        wp_sb[:, ct, ocs],
                        start=(ct == 0), stop=(ct == 7),
                    )
                # bias applied during the PSUM->SBUF copy (DVE, not PE)
                o_sb = osbp.tile([128, 512], F32, tag="osb")
                nc.vector.tensor_tensor(
                    o_sb[:], o_ps[:], bprime_bc[:, ocs], mybir.AluOpType.add
                )
                for qtr in range(2):
                    qs = slice(qtr * 256, (qtr + 1) * 256)
                    nc.sync.dma_start(
                        out_r[:, b * 2 + slot, oc * 512 + qtr * 256 : oc * 512 + (qtr + 1) * 256],
                        o_sb[:, qs],
                    )

            # ================= fused pipeline =================
            # chunks are issued one qkv r-chunk behind the one that feeds
            # them, so DVE copy-outs (qT/kT/vp) land well before the PE
            # reads them
            # qkv chunks run >=2 chunks ahead of the attn pair that
            # consumes them: the PE pulls LDWEIGHTS ahead of in-flight
            # matmuls (64-deep reorder window), so the DVE writes of
            # qT/kT/vp need real slack before the PE's weight fetch —
            # a bare semaphore is not enough on hardware.
            # proj(b-1) is issued after fire_cc(b) so its PE work hides
            # the collective; batch 3 runs its q-chunks largest-first so
            # the kernel tail ends on the smallest ACT/exp chunk.
            for b in range(3):
                qkv_r(b, 0)
                if b == 0:
                    late_consts()
                qkv_r(b, 1)
                qkv_r(b, 2)
                attn_pair(b, 0)
                qkv_r(b, 3)
                attn_pair(b, 1)
                attn_pair(b, 2)
                attn_pair(b, 3)
                fire_cc(b)
                if b >= 1:
                    yth_prev = yth_load(b - 1)
                    for gi in range(4):
                        proj_group(b - 1, yth_prev, gi)

            qkv_r(3, 0)
            qkv_r(3, 1)
            qkv_r(3, 2)
            qkv_r(3, 3)
            yth2 = yth_load(2)
            proj_group(2, yth2, 0)
            proj_group(2, yth2, 1)
            attn_pair(3, 3)
            attn_pair(3, 2)
            attn_pair(3, 1)
            attn_pair(3, 0)
            fire_cc(3)
            proj_group(2, yth2, 2)
            proj_group(2, yth2, 3)

            yth3 = yth_load(3)
            for gi in range(4):
                proj_group(3, yth3, gi)

            if dbg:
                nc.sync.dma_start(dbg_bpc[:], bprime_bc[:])
                for sl in range(2):
                    cs = slice(sl * 128, (sl + 1) * 128)
                    nc.sync.dma_start(dbg_ain[:, cs], a2a_in[0][sl][:])
                    nc.sync.dma_start(dbg_aout[:, cs], a2a_out[0][sl][:])
                nc.sync.dma_start(dbg_yth.rearrange("p (ct q) -> p ct q", ct=8), yth3[:])
                nc.sync.dma_start(dbg_q[:], qT_sb[:])
                nc.sync.dma_start(dbg_k[:], kT_sb[:])
                nc.sync.dma_start(
                    dbg_vp.rearrange("p (s c) -> p s c", c=256), vp_sb[:]
                )

    nc.finalize()
    return nc


def _prep_inputs(x, c_attn_w, c_attn_b, c_proj_w, c_proj_b):
    x = np.asarray(x, dtype=np.float32)
    c_attn_w = np.asarray(c_attn_w, dtype=np.float32)
    c_attn_b = np.asarray(c_attn_b, dtype=np.float32)
    c_proj_w = np.asarray(c_proj_w, dtype=np.float32)
    c_proj_b = np.asarray(c_proj_b, dtype=np.float32)

    xT = np.ascontiguousarray(x.reshape(ROWS, C).T).astype(BF16)
    wq, wk, wv_full = c_attn_w[:, :C], c_attn_w[:, C : 2 * C], c_attn_w[:, 2 * C :]
    bqf, bkf, bvf = c_attn_b[:C], c_attn_b[C : 2 * C], c_attn_b[2 * C :]
    wp_b = np.ascontiguousarray(c_proj_w).astype(BF16)
    bprime = (bvf @ c_proj_w + c_proj_b).reshape(1, C).astype(np.float32)
    maskb = np.tril(np.full((128, 128), -30.0, dtype=np.float32), -1).astype(BF16)
    ident = np.eye(128, dtype=np.float32).astype(BF16)

    in_maps = []
    for c in range(NCORES):
        cs = slice(c * CPC, (c + 1) * CPC)
        in_maps.append(
            {
                "xT": xT,
                "wqk": np.ascontiguousarray(
                    np.concatenate([wq[:, cs], wk[:, cs]], axis=1)
                ).astype(BF16),
                "wv": np.ascontiguousarray(wv_full[:, cs]).astype(BF16),
                "bq": np.ascontiguousarray(bqf[cs].reshape(CPC, 1)).astype(np.float32),
                "bk": np.ascontiguousarray(bkf[cs].reshape(CPC, 1)).astype(np.float32),
                "wp": wp_b,
                "bprime": bprime,
                "maskb": maskb,
                "ident": ident,
            }
        )
    return in_maps


def _unshard(results):
    # core c, rows b*256 + i  ->  full row b*2048 + c*256 + i
    stk = np.stack([results[c]["out"] for c in range(NCORES)])  # [8,1024,C]
    full = stk.reshape(NCORES, B, 256, C).transpose(1, 0, 2, 3)
    return np.ascontiguousarray(full.reshape(B, T, C)).astype(np.float32)


def kernel(x, c_attn_w, c_attn_b, c_proj_w, c_proj_b):
    from concourse.bass_utils import run_bass_kernel_spmd

    if "nc" not in _CACHE:
        _CACHE["nc"] = _build()
    nc = _CACHE["nc"]

    in_maps = _prep_inputs(x, c_attn_w, c_attn_b, c_proj_w, c_proj_b)
    if "warm" not in _CACHE:
        # first execution in a process can race device-side one-time init
        # (gpsimd library residency); throw one run away
        run_bass_kernel_spmd(nc, in_maps, core_ids=list(range(NCORES)))
        _CACHE["warm"] = True
    res = run_bass_kernel_spmd(nc, in_maps, core_ids=list(range(NCORES)))
    return _unshard(res.results)


# revision 14
# speedup vs baseline: 1.1110x; 1.0830x over previous
"""Causal self-attention (B=4, T=2048, C=1024, H=16) on 8 trn2 NeuronCores.

Sharding: tensor-parallel over heads (2 heads/core) for QKV projection +
attention; output rows are resharded core-cyclically by 128-row tiles
(dest core d owns row-tiles t with t % 8 == d) so each batch gets its own
quarter-size AllToAll that serves all 8 cores symmetrically — the
collective for batch b fires as soon as batch b's attention is done and
overlaps batch b+1's compute.  Host gather interleaves the tiles back.

Key PE optimizations over the plain per-head schedule:
  * QK^T is ROW-TILED: head 0 lives on partitions 0-63 of the qT/kT
    slabs, head 1 on 64-127.  Each k-tile issues two 64-contraction
    matmuls at tile_position (0,0) and (64,0) which execute
    CONCURRENTLY on the two halves of the PE array (~2x QK throughput).
    Scores share one [128,1024] PSUM group (h0 cols 0:512, h1 512:1024)
    so a single ACT exp instruction covers both heads.
  * V is computed directly in [row, oc] layout (x-stationary matmuls)
    — no PE transposes, no vst staging copy.
  * V slots are packed [v_h0(64) | ones | v_h1(64) | ones] so the PV
    matmul of either head is a 65-wide lhsT whose output carries y in
    partitions 0-63 and the softmax denominator in partition 64 (no
    max-subtraction: logits are small).  No zero padding needed.
  * c_proj bias is applied by the DVE during the PSUM->SBUF copy
    (tensor_tensor add against a gpsimd-broadcast bias tile) instead of
    a PE matmul.
The causal mask stays a -30 additive bias via PE matmul accumulation
(identity x mask-bias) on the two diagonal-tile column blocks.
The normalizer 1/den is partition-broadcast on gpsimd; y/den are copied
out of PSUM in one [65,512] DVE op so the bank frees early.

PSUM budget (8 banks): score groups 2x2 + y accum 2x1 + shared
qkv/proj accumulator 2x1.
"""

import sys

for _p in ("/opt/trn_rl_repo",):
    if _p not in sys.path:
        sys.path.insert(0, _p)

import numpy as np
import ml_dtypes

B, T, C, H, HS = 4, 2048, 1024, 16, 64
NCORES = 8
HPC = H // NCORES            # heads per core = 2
CPC = HPC * HS               # channels per core = 128
ROWS = B * T                 # 8192
RPC = ROWS // NCORES         # rows per core = 1024
NKT = T // 128               # k-tiles per batch = 16

BF16 = ml_dtypes.bfloat16

_CACHE: dict = {}


def _apply_tile_tail_patch(tile_mod):
    """This container's walrus rejects CTRL-class instructions (Drain/NoOp)
    carrying semaphore waits. Re-emit TileContext's tail waits as individual
    EventSemaphore waits and use the sem-only barrier variant."""
    import bass_rust
    from concourse.vector_clock import ScopedClock

    if getattr(tile_mod.TileContext, "_tail_patch_applied", False):
        return

    def _drain_and_barrier(self, tick_clock, wait_clock):
        collector = self.nc.sync.nop(nofuse=True, hint="tile_tail_wait")
        wait_clock.add_sem_waits(
            collector.ins, ScopedClock({None: tick_clock.global_clock})
        )
        si = collector.ins.sync_info
        waits = list(si.on_wait) if si is not None else []
        collector.ins.sync_info = None
        for w in waits:
            assert w.wait_mode == "sem-ge-imm", w
            self.nc.sync.wait_ge(
                bass_rust.SemaphoreHandle(w.ant_name, w.id), w.wait_value
            )

        self.nc.all_engine_barrier(sem_only=True)
        assert self.sems is not None
        popped = self.nc._tile_sem_poison_stack.pop()
        assert popped is self._sem_poison
        self.nc.clear_and_free_semaphores(list(self.sems.allocated().values()))
        self.nc.all_engine_barrier(sem_only=True)

    tile_mod.TileContext._drain_and_barrier = _drain_and_barrier
    tile_mod.TileContext._tail_patch_applied = True


def _build(dbg=False):
    import concourse.bass as bass
    import concourse.bacc as bacc
    import concourse.mybir as mybir
    import concourse.tile as tile

    _apply_tile_tail_patch(tile)

    dt = mybir.dt
    F32 = dt.float32
    BF = dt.bfloat16
    Exp = mybir.ActivationFunctionType.Exp

    nc = bacc.Bacc(num_devices=NCORES)

    if dbg:
        dbg_q = nc.dram_tensor("dbg_q", [128, ROWS], BF, kind="ExternalOutput")
        dbg_k = nc.dram_tensor("dbg_k", [128, ROWS], BF, kind="ExternalOutput")
        dbg_vp = nc.dram_tensor("dbg_vp", [128, NKT * B * 256], BF, kind="ExternalOutput")
        dbg_ycp = nc.dram_tensor("dbg_ycp", [65, 32, 512], F32, kind="ExternalOutput")
        dbg_bpc = nc.dram_tensor("dbg_bpc", [128, C], F32, kind="ExternalOutput")
        dbg_ain = nc.dram_tensor("dbg_ain", [NCORES * CPC, 256], BF, kind="ExternalOutput")
        dbg_aout = nc.dram_tensor("dbg_aout", [NCORES * CPC, 256], BF, kind="ExternalOutput")
        dbg_yth = nc.dram_tensor("dbg_yth", [128, 8 * 256], BF, kind="ExternalOutput")

    xT = nc.dram_tensor("xT", [C, ROWS], BF, kind="ExternalInput")
    wqk = nc.dram_tensor("wqk", [C, 2 * CPC], BF, kind="ExternalInput")
    wv = nc.dram_tensor("wv", [C, CPC], BF, kind="ExternalInput")
    bq = nc.dram_tensor("bq", [CPC, 1], F32, kind="ExternalInput")
    bk = nc.dram_tensor("bk", [CPC, 1], F32, kind="ExternalInput")
    wp = nc.dram_tensor("wp", [C, C], BF, kind="ExternalInput")
    bprime = nc.dram_tensor("bprime", [1, C], F32, kind="ExternalInput")
    maskb = nc.dram_tensor("maskb", [128, 128], BF, kind="ExternalInput")
    ident = nc.dram_tensor("ident", [128, 128], BF, kind="ExternalInput")
    out = nc.dram_tensor("out", [RPC, C], F32, kind="ExternalOutput")

    with tile.TileContext(nc) as tc:
        with (
            tc.tile_pool(name="const", bufs=1) as constp,
            tc.tile_pool(name="big", bufs=1) as bigp,
            tc.tile_pool(name="xin", bufs=3) as xinp,
            tc.tile_pool(name="pt", bufs=3) as ptp,
            tc.tile_pool(name="tail", bufs=4) as tailp,
            tc.tile_pool(name="osb", bufs=2) as osbp,
            tc.tile_pool(name="yth", bufs=2) as ythp,
            tc.tile_pool(name="psy", bufs=2, space="PSUM") as psy,
            tc.tile_pool(name="psst", bufs=2, space="PSUM") as psst,
            tc.tile_pool(name="psmm", bufs=2, space="PSUM") as psmm,
            tc.tile_pool(name="dram", bufs=1, space="DRAM") as dramp,
        ):
            # ---- constants (only what the first r-chunk needs up front;
            # the rest is issued after the first x tile DMA so PE starts
            # ~20us earlier) ----
            # split per contraction-tile so the transfers round-robin over
            # DMA queues and the first q matmul starts ~3us in
            wqk_sb = constp.tile([128, 8, 2 * CPC], BF, tag="wqk")
            wqk_r = wqk.rearrange("(ct p) o -> p ct o", p=128)
            for ct in range(8):
                nc.sync.dma_start(wqk_sb[:, ct, :], wqk_r[:, ct, :])
            bq_sb = constp.tile([CPC, 1], F32, tag="bq")
            nc.sync.dma_start(bq_sb[:], bq[:])
            bk_sb = constp.tile([CPC, 1], F32, tag="bk")
            nc.sync.dma_start(bk_sb[:], bk[:])
            wv_sb = constp.tile([128, 8, CPC], BF, tag="wv")
            wv_r = wv.rearrange("(ct p) o -> p ct o", p=128)
            for ct in range(8):
                nc.sync.dma_start(wv_sb[:, ct, :], wv_r[:, ct, :])
            wp_sb = constp.tile([128, 8, C], BF, tag="wp")
            bprime_sb = constp.tile([1, C], F32, tag="bprime")
            bprime_bc = constp.tile([128, C], F32, tag="bprime_bc")
            maskb_sb = constp.tile([128, 128], BF, tag="maskb")
            nc.sync.dma_start(maskb_sb[:], maskb[:])
            ident_sb = constp.tile([128, 128], BF, tag="ident")
            nc.sync.dma_start(ident_sb[:], ident[:])

            # dummy broadcast: forces the gpsimd Q7 library resident before
            # the first real per-chunk broadcast (~15us in) — the library
            # load otherwise races it on the first execution in a process
            warm_src = constp.tile([1, 512], F32, tag="wsrc")
            nc.vector.memset(warm_src[:], 1.0)
            warm_dst = constp.tile([64, 512], F32, tag="wdst")
            nc.gpsimd.partition_broadcast(warm_dst[:], warm_src[:])

            def late_consts():
                nc.sync.dma_start(bprime_sb[:], bprime[:])
                nc.gpsimd.partition_broadcast(bprime_bc[:], bprime_sb[:])
                nc.sync.dma_start(
                    wp_sb[:], wp.rearrange("(ct p) o -> p ct o", p=128)
                )

            # ---- persistent intermediates ----
            # qT/kT: [128, ROWS] — head 0 channels on partitions 0-63,
            # head 1 on 64-127 (the row-tiling layout).
            qT_sb = bigp.tile([128, ROWS], BF, tag="qT")
            kT_sb = bigp.tile([128, ROWS], BF, tag="kT")
            # v' per global k-tile: [128 rows, 64 slots, 2*128]; per
            # head half: [v 64 | ones | 63 zeros] (the 65-col lhsT read
            # produced garbage on hw; 128-col weights are the safe path).
            vp_sb = bigp.tile([128, NKT * B, 256], BF, tag="vp")
            vpr = vp_sb[:].rearrange("p s (h c) -> p s h c", c=128)
            nc.vector.memset(vpr[:, :, :, 64:65], 1.0)
            nc.vector.memset(vpr[:, :, :, 65:128], 0.0)

            # per-(batch, slot) AllToAll buffers: [8 dests x 128ch, 128 q]
            a2a_in = [
                [
                    dramp.tile([NCORES * CPC, 128], BF, name=f"a2a_in{b}_{sl}")
                    for sl in range(2)
                ]
                for b in range(B)
            ]
            a2a_out = [
                [
                    dramp.tile([NCORES * CPC, 128], BF, name=f"a2a_out{b}_{sl}")
                    for sl in range(2)
                ]
                for b in range(B)
            ]

            xT_r = xT.rearrange("(ct p) r -> p ct r", p=128)

            def qkv_r(b, rb):
                with nc.named_scope(f"qkv{b}{rb}"):
                    _qkv_r(b, rb)

            def _qkv_r(b, rb):
                r = b * 4 + rb
                rs = slice(r * 512, (r + 1) * 512)
                xt = xinp.tile([128, 8, 512], BF, tag="xt")
                for cth in range(4):
                    nc.sync.dma_start(
                        xt[:, 2 * cth : 2 * cth + 2, :],
                        xT_r[:, 2 * cth : 2 * cth + 2, rs],
                    )

                q_ps = psmm.tile([128, 512], F32, tag="mm", name=f"qps_{r}")
                for ct in range(8):
                    nc.tensor.matmul(
                        q_ps[:], wqk_sb[:, ct, 0:CPC], xt[:, ct, :],
                        start=(ct == 0), stop=(ct == 7),
                    )
                k_ps = psmm.tile([128, 512], F32, tag="mm", name=f"kps_{r}")
                for ct in range(8):
                    nc.tensor.matmul(
                        k_ps[:], wqk_sb[:, ct, CPC:], xt[:, ct, :],
                        start=(ct == 0), stop=(ct == 7),
                    )
                # single fused bias(+scale) copies: partitions 0-63 are
                # head 0's 64 dims, 64-127 head 1's (matches row tiling)
                nc.vector.tensor_scalar(
                    qT_sb[:, rs], q_ps[:], bq_sb[:], 0.125,
                    mybir.AluOpType.add, mybir.AluOpType.mult,
                )
                nc.vector.tensor_scalar(
                    kT_sb[:, rs], k_ps[:], bk_sb[:], None,
                    mybir.AluOpType.add,
                )
                # v computed directly in [row, oc] layout: x-chunk
                # stationary, wv moving — no transposes needed
                v_ps = psmm.tile([128, 512], F32, tag="mm", name=f"vps_{r}")
                for t in range(4):
                    for ct in range(8):
                        nc.tensor.matmul(
                            v_ps[:, t * 128 : (t + 1) * 128],
                            xt[:, ct, t * 128 : (t + 1) * 128],
                            wv_sb[:, ct, :],
                            start=(ct == 0), stop=(ct == 7),
                        )
                for t in range(4):
                    slot = 4 * r + t
                    # one strided copy: [128,2,64] src (heads side by side)
                    # -> dst cols {0:64, 128:192} (stride 128)
                    nc.vector.tensor_copy(
                        out=vp_sb[:, slot, 0:256].rearrange(
                            "p (h c) -> p h c", c=128
                        )[:, :, 0:64],
                        in_=v_ps[:, t * 128 : (t + 1) * 128].rearrange(
                            "p (h c) -> p h c", c=64
                        ),
                    )

            def attn_pair(b, qc):
                with nc.named_scope(f"at{b}{qc}"):
                    _attn_pair(b, qc)

            def _attn_pair(b, qc):
                q0 = qc * 512
                nkt = 4 * qc + 4
                y_ps = [
                    psy.tile([128, 512], F32, tag="y", name=f"y_{b}_{h}_{qc}")
                    for h in range(2)
                ]

                def qk_tile(kt):
                    n = 512 - max(0, (kt - 4 * qc) * 128)
                    diag = kt >= 4 * qc
                    st = psst.tile(
                        [128, 1024], F32, tag="st", name=f"st_{b}_{qc}_{kt}"
                    )
                    ks = slice(b * T + kt * 128, b * T + (kt + 1) * 128)
                    qs = slice(b * T + q0 + 512 - n, b * T + q0 + 512)
                    # two 64-contraction matmuls on PE row-tiles (0,0) and
                    # (64,0) — they run concurrently
                    nc.tensor.matmul(
                        st[:, 0:n], kT_sb[0:64, ks], qT_sb[0:64, qs],
                        start=True, stop=not diag,
                    )
                    nc.tensor.matmul(
                        st[:, 512 : 512 + n], kT_sb[64:128, ks], qT_sb[64:128, qs],
                        start=True, stop=not diag,
                    )
                    if diag:
                        nc.tensor.matmul(
                            st[:, 0:128], ident_sb[:], maskb_sb[:],
                            start=False, stop=True,
                        )
                        nc.tensor.matmul(
                            st[:, 512:640], ident_sb[:], maskb_sb[:],
                            start=False, stop=True,
                        )
                    pT = ptp.tile([128, 1024], BF, tag="pT")
                    if n == 512:
                        nc.scalar.activation(pT[:], st[:], Exp)
                    else:
                        # strided [128, 2, n] view skips the stale middle
                        nc.scalar.activation(
                            pT[:].rearrange("p (h c) -> p h c", c=512)[:, :, 0:n],
                            st[:].rearrange("p (h c) -> p h c", c=512)[:, :, 0:n],
                            Exp,
                        )
                    return pT, n

                def pv_tile(kt, pT, n):
                    slot = b * NKT + kt
                    for h in range(2):
                        nc.tensor.matmul(
                            y_ps[h][:, 512 - n :],
                            vp_sb[:, slot, 128 * h : 128 * h + 128],
                            pT[:, 512 * h : 512 * h + n],
                            start=(kt == 0),
                            stop=(kt == nkt - 1),
                        )

                pend = [qk_tile(0), qk_tile(1)]
                for kt in range(nkt):
                    args = pend.pop(0)
                    if kt + 2 < nkt:
                        pend.append(qk_tile(kt + 2))
                    pv_tile(kt, *args)

                # tails: copy y & den out of PSUM in one op (frees the
                # bank), then normalize via gpsimd broadcast of 1/den
                for h in range(2):
                    ycp = tailp.tile([65, 512], F32, tag="ycp")
                    nc.vector.tensor_copy(out=ycp[:], in_=y_ps[h][0:65, :])
                    # den must land at partition base 0 before the custom-DVE
                    # reciprocal (cross-partition-base input misreads there)
                    den = tailp.tile([1, 512], F32, tag="den")
                    nc.vector.tensor_copy(out=den[:], in_=y_ps[h][64:65, :])
                    rcp = tailp.tile([1, 512], F32, tag="rcp")
                    nc.vector.reciprocal_approx_fast(rcp[:], den[:])
                    bc = tailp.tile([64, 512], F32, tag="bc")
                    nc.gpsimd.partition_broadcast(bc[:], rcp[:])
                    yT = tailp.tile([64, 512], BF, tag="yT")
                    nc.vector.tensor_tensor(
                        yT[:], ycp[0:64, :], bc[:], mybir.AluOpType.mult
                    )
                    if dbg:
                        nc.sync.dma_start(dbg_ycp[:, b * 8 + qc * 2 + h, :], ycp[:])
                    # scatter to the 4 dest cores' slots of this batch's a2a
                    dst = a2a_in[b][qc // 2][:].rearrange(
                        "(d ch) q -> ch d q", ch=128
                    )[h * 64 : (h + 1) * 64, 4 * (qc % 2) : 4 * (qc % 2) + 4, :]
                    nc.sync.dma_start(
                        dst, yT[:].rearrange("p (s q) -> p s q", q=128)
                    )

            def fire_cc(b, sl):
                nc.gpsimd.collective_compute(
                    "AllToAll",
                    mybir.AluOpType.bypass,
                    replica_groups=[list(range(NCORES))],
                    ins=[a2a_in[b][sl][:].opt()],
                    outs=[a2a_out[b][sl][:].opt()],
                )

            def yth_load(b):
                yth = ythp.tile([128, 8, 256], BF, tag="yth", name=f"yth{b}")
                for sl in range(2):
                    nc.sync.dma_start(
                        yth[:, :, sl * 128 : (sl + 1) * 128],
                        a2a_out[b][sl][:].rearrange("(ct p) q -> p ct q", p=128),
                    )
                return yth

            out_r = out.rearrange("(bt p) o -> p bt o", p=128)

            def proj_group(b, yth, gi):
                with nc.named_scope(f"pj{b}{gi}"):
                    _proj_group(b, yth, gi)

            def _proj_group(b, yth, gi):
                slot, oc = gi // 2, gi % 2
                ocs = slice(oc * 512, (oc + 1) * 512)
                o_ps = psmm.tile([128, 512], F32, tag="mm", name=f"o_{b}_{gi}")
                for ct in range(8):
                    nc.tensor.matmul(
                        o_ps[:],
                        yth[:, ct, slot * 128 : (slot + 1) * 128],
                        wp_sb[:, ct, ocs],
                        start=(ct == 0), stop=(ct == 7),
                    )
                # bias applied during the PSUM->SBUF copy (DVE, not PE)
                o_sb = osbp.tile([128, 512], F32, tag="osb")
                nc.vector.tensor_tensor(
                    o_sb[:], o_ps[:], bprime_bc[:, ocs], mybir.AluOpType.add
                )
                for qtr in range(2):
                    qs = slice(qtr * 256, (qtr + 1) * 256)
                    nc.sync.dma_start(
                        out_r[:, b * 2 + slot, oc * 512 + qtr * 256 : oc * 512 + (qtr + 1) * 256],
                        o_sb[:, qs],
                    )

            # ================= fused pipeline =================
            # chunks are issued one qkv r-chunk behind the one that feeds
            # them, so DVE copy-outs (qT/kT/vp) land well before the PE
            # reads them
            # qkv chunks run >=2 chunks ahead of the attn pair that
            # consumes them: the PE pulls LDWEIGHTS ahead of in-flight
            # matmuls (64-deep reorder window), so the DVE writes of
            # qT/kT/vp need real slack before the PE's weight fetch —
            # a bare semaphore is not enough on hardware.
            for b in range(B):
                qkv_r(b, 0)
                if b == 0:
                    late_consts()
                qkv_r(b, 1)
                qkv_r(b, 2)
                attn_pair(b, 0)
                qkv_r(b, 3)
                attn_pair(b, 1)
                fire_cc(b, 0)
                attn_pair(b, 2)
                attn_pair(b, 3)
                if b >= 1:
                    yth_prev = yth_load(b - 1)
                    for gi in range(4):
                        proj_group(b - 1, yth_prev, gi)
                fire_cc(b, 1)

            yth3 = yth_load(3)
            for gi in range(4):
                proj_group(3, yth3, gi)

            if dbg:
                nc.sync.dma_start(dbg_bpc[:], bprime_bc[:])
                for sl in range(2):
                    cs = slice(sl * 128, (sl + 1) * 128)
                    nc.sync.dma_start(dbg_ain[:, cs], a2a_in[0][sl][:])
                    nc.sync.dma_start(dbg_aout[:, cs], a2a_out[0][sl][:])
                nc.sync.dma_start(dbg_yth.rearrange("p (ct q) -> p ct q", ct=8), yth3[:])
                nc.sync.dma_start(dbg_q[:], qT_sb[:])
                nc.sync.dma_start(dbg_k[:], kT_sb[:])
                nc.sync.dma_start(
                    dbg_vp.rearrange("p (s c) -> p s c", c=256), vp_sb[:]
                )

    nc.finalize()
    return nc


def _prep_inputs(x, c_attn_w, c_attn_b, c_proj_w, c_proj_b):
    x = np.asarray(x, dtype=np.float32)
    c_attn_w = np.asarray(c_attn_w, dtype=np.float32)
    c_attn_b = np.asarray(c_attn_b, dtype=np.float32)
    c_proj_w = np.asarray(c_proj_w, dtype=np.float32)
    c_proj_b = np.asarray(c_proj_b, dtype=np.float32)

    xT = np.ascontiguousarray(x.reshape(ROWS, C).T).astype(BF16)
    wq, wk, wv_full = c_attn_w[:, :C], c_attn_w[:, C : 2 * C], c_attn_w[:, 2 * C :]
    bqf, bkf, bvf = c_attn_b[:C], c_attn_b[C : 2 * C], c_attn_b[2 * C :]
    wp_b = np.ascontiguousarray(c_proj_w).astype(BF16)
    bprime = (bvf @ c_proj_w + c_proj_b).reshape(1, C).astype(np.float32)
    maskb = np.tril(np.full((128, 128), -30.0, dtype=np.float32), -1).astype(BF16)
    ident = np.eye(128, dtype=np.float32).astype(BF16)

    in_maps = []
    for c in range(NCORES):
        cs = slice(c * CPC, (c + 1) * CPC)
        in_maps.append(
            {
                "xT": xT,
                "wqk": np.ascontiguousarray(
                    np.concatenate([wq[:, cs], wk[:, cs]], axis=1)
                ).astype(BF16),
                "wv": np.ascontiguousarray(wv_full[:, cs]).astype(BF16),
                "bq": np.ascontiguousarray(bqf[cs].reshape(CPC, 1)).astype(np.float32),
                "bk": np.ascontiguousarray(bkf[cs].reshape(CPC, 1)).astype(np.float32),
                "wp": wp_b,
                "bprime": bprime,
                "maskb": maskb,
                "ident": ident,
            }
        )
    return in_maps


def _unshard(results):
    # core c, rows b*256 + slot*128 + i  ->  full row b*2048 + (slot*8+c)*128 + i
    stk = np.stack([results[c]["out"] for c in range(NCORES)])  # [8,1024,C]
    full = stk.reshape(NCORES, B, 2, 128, C).transpose(1, 2, 0, 3, 4)
    return np.ascontiguousarray(full.reshape(B, T, C)).astype(np.float32)


def kernel(x, c_attn_w, c_attn_b, c_proj_w, c_proj_b):
    from concourse.bass_utils import run_bass_kernel_spmd

    if "nc" not in _CACHE:
        _CACHE["nc"] = _build()
    nc = _CACHE["nc"]

    in_maps = _prep_inputs(x, c_attn_w, c_attn_b, c_proj_w, c_proj_b)
    if "warm" not in _CACHE:
        # first execution in a process can race device-side one-time init
        # (gpsimd library residency); throw one run away
        run_bass_kernel_spmd(nc, in_maps, core_ids=list(range(NCORES)))
        _CACHE["warm"] = True
    res = run_bass_kernel_spmd(nc, in_maps, core_ids=list(range(NCORES)))
    return _unshard(res.results)
